# revision 1
# baseline (speedup 1.0000x reference)
"""Causal KV-attention Trainium2 kernel (Bass/Tile), SPMD over 8 NeuronCores.

Problem: B=4, L=4096, E=512 fp32.
  q = x@Wq.T + bq ; k = x@Wk.T + bk ; v = x@Wv.T + bv
  out = softmax(causal_mask(q@k.T)/sqrt(E)) @ v

Sharding: one core = (batch element, half of the queries). Query rows are
split into 256-row chunks; variant A takes chunks paired with variant B so
total causal work is balanced. All 8 cores run the SAME program (SPMD);
per-core differences are carried entirely in data:
  - xq: this core's query rows, gathered into schedule order.
  - cvec: per (chunk-position, tail-block) thresholds from which the device
    builds the additive causal masks (0/-1e9) with an iota + compare.

All per-core inputs are packed into ONE f32 blob. On the fast path the blob
is assembled ON DEVICE by a small XLA program (pair-wise ppermute of batch
halves + weight all-gather + query-row take), so the host only uploads the
unique 35MB instead of 8 full per-core copies; host-side blob assembly is
kept as a fallback.

On-chip dataflow (per core):
  x tiles are PE-transposed to xT; K^T,Q^T in [E,L] layout and V in [L,E]
  layout via fp32r matmuls. Attention processes 256-query chunks: S^T[k,q]
  blocks come straight out of the PE (no P-transposes), exp on ACT, then
  P^T blocks feed O = P@V as stationary operands with V streaming. Row
  softmax denominators come from an extra N=2 matmul against ones.
"""

import math
import os

import numpy as np

os.environ.setdefault("NEURON_RT_RESET_CORES", "1")

P = 128
E = 512
EB = E // P  # 4 e-blocks
QC = 256  # query chunk width
SCALE = 1.0 / math.sqrt(E)
NEG = -1.0e9

_CACHE = {}
DYNQ = False  # gather Q rows on device via dynamic DMA (unsupported under axon PJRT)


# ---------------------------------------------------------------------------
# host-side layout
# ---------------------------------------------------------------------------
def _chunk_layout(L):
    """Returns (chunks_A, chunks_B, KB).

    Query rows are split into n=L/256 chunks; chunk t needs 2t+2 key blocks
    (128 keys each). Variant A owns {t<n/2 even} + {t>=n/2 with n-1-t even},
    B the complement; each variant's chunks are sorted by descending block
    count and padded to the common schedule KB[i] = 2n - 4i.
    """
    n = L // QC
    a = [t for t in range(n // 2) if t % 2 == 0] + [
        t for t in range(n // 2, n) if (n - 1 - t) % 2 == 0
    ]
    b = [t for t in range(n) if t not in a]
    key = lambda t: -(2 * t + 2)
    a = sorted(a, key=key)
    b = sorted(b, key=key)
    KB = [2 * n - 4 * i for i in range(n // 2)]
    for i in range(n // 2):
        assert KB[i] >= max(2 * a[i] + 2, 2 * b[i] + 2) and KB[i] >= 4
        assert KB[i] * P >= QC * a[i] + QC and KB[i] * P >= QC * b[i] + QC
    return a, b, KB


def _blob_spec(L):
    """(name -> (offset_words, shape)) for the packed per-core input blob."""
    n_pos = (L // QC) // 2
    Lq = QC * n_pos
    spec = {}
    off = 0
    for name, shape in (
        ("xk", (L, E)),
        ("wq", (E, E)),
        ("wk", (E, E)),
        ("wv", (E, E)),
        ("bq", (E,)),
        ("bk", (E,)),
        ("bv", (E,)),
        ("cvec", (n_pos * 4,)),
        ("qoff", (Lq // P,)),  # int32 bits in f32 storage
    ) + (() if DYNQ else (("xq", (Lq, E)),)):
        spec[name] = (off, shape)
        off += int(np.prod(shape))
    return spec, off


def _shard_inputs(x, Wq, bq, Wk, bk, Wv, bv, L):
    ca, cb, KB = _chunk_layout(L)
    n_pos = len(KB)
    spec, words = _blob_spec(L)

    def cvec_for(chunks):
        c = np.zeros((n_pos, 4), np.float32)
        for i, t in enumerate(chunks):
            for s in range(4):
                kb = KB[i] - 4 + s
                c[i, s] = float(128 * kb - QC * t)  # masked iff (j - p) < c
        return c.reshape(-1)

    def qoff_for(chunks):
        q = np.zeros(len(chunks) * 2, np.int32)
        for i, t in enumerate(chunks):
            q[2 * i] = QC * t
            q[2 * i + 1] = QC * t + P
        return q.view(np.float32)

    cv = {0: cvec_for(ca), 1: cvec_for(cb)}
    qo = {0: qoff_for(ca), 1: qoff_for(cb)}
    blobs = np.empty((8, words), np.float32)
    for c in range(8):
        b, v = c // 2, c % 2
        parts = {
            "xk": np.asarray(x[b], np.float32),
            "wq": np.asarray(Wq, np.float32),
            "wk": np.asarray(Wk, np.float32),
            "wv": np.asarray(Wv, np.float32),
            "bq": np.asarray(bq, np.float32),
            "bk": np.asarray(bk, np.float32),
            "bv": np.asarray(bv, np.float32),
            "cvec": cv[v],
            "qoff": qo[v],
        }
        if not DYNQ:
            chunks = ca if v == 0 else cb
            parts["xq"] = np.concatenate(
                [np.asarray(x[b], np.float32)[QC * t : QC * t + QC] for t in chunks]
            )
        for name, (off, shape) in spec.items():
            n = int(np.prod(shape))
            blobs[c, off : off + n] = parts[name].reshape(-1)
    return [{"blob": blobs[c]} for c in range(8)], (ca, cb)


def _gather_outputs(results, ca_cb, B, L):
    ca, cb = ca_cb
    y = np.empty((B, L, E), np.float32)
    for c in range(8):
        b, v = c // 2, c % 2
        chunks = ca if v == 0 else cb
        yq = results[c]["yq"]
        for i, t in enumerate(chunks):
            y[b, QC * t : QC * t + QC] = yq[QC * i : QC * i + QC]
    return y


# ---------------------------------------------------------------------------
# device program
# ---------------------------------------------------------------------------
def build_program(L=4096):
    from contextlib import ExitStack

    import concourse.bass as bass
    import concourse.mybir as mybir
    import concourse.tile as tile
    from concourse import bacc
    from concourse.bass import ds
    from concourse.masks import make_identity

    f32 = mybir.dt.float32
    f32r = mybir.dt.float32r
    i32 = mybir.dt.int32
    Exp = mybir.ActivationFunctionType.Exp
    Ident = mybir.ActivationFunctionType.Identity

    n_chunks = L // QC
    n_pos = n_chunks // 2
    Lq = QC * n_pos  # queries per core
    NKB = L // P  # total key blocks
    NCH = L // 512  # 512-row l-chunks of the key rows
    _, _, KB = _chunk_layout(L)
    spec, words = _blob_spec(L)

    nc = bacc.Bacc("TRN2", target_bir_lowering=False, debug=False, num_devices=8)

    blob_d = nc.dram_tensor("blob", [words], f32, kind="ExternalInput").ap()
    yq_d = nc.dram_tensor("yq", [Lq, E], f32, kind="ExternalOutput").ap()

    def bpart(name):
        off, shape = spec[name]
        n = int(np.prod(shape))
        p = blob_d[off : off + n]
        if len(shape) == 2:
            p = p.rearrange("(r c) -> r c", c=shape[1])
        return p

    xk_d = bpart("xk")
    w_d = {"wq": bpart("wq"), "wk": bpart("wk"), "wv": bpart("wv")}
    cvec_d = bpart("cvec")
    qoff_d = bpart("qoff")

    with ExitStack() as ctx:
        tc = ctx.enter_context(tile.TileContext(nc))

        const = ctx.enter_context(tc.tile_pool(name="const", bufs=1))
        big = ctx.enter_context(tc.tile_pool(name="big", bufs=1))

        # --- constants ---
        ident = const.tile([P, P], f32, tag="ident", name="ident")
        make_identity(nc, ident)
        ones_f = const.tile([P, 2], f32, tag="ones_f", name="ones_f")
        nc.vector.memset(ones_f, 1.0)
        ones = const.tile([P, 2], f32r, tag="ones", name="ones")
        nc.vector.tensor_copy(out=ones, in_=ones_f)
        bv_bc = const.tile([P, E], f32, tag="bv_bc", name="bv_bc")
        nc.gpsimd.dma_start(
            out=bv_bc,
            in_=bass.AP(
                tensor=blob_d.tensor, offset=spec["bv"][0], ap=[[0, P], [1, E]]
            ),
        )
        bias_t = {}
        for nm in ("bq", "bk"):
            for eb in range(EB):
                t = const.tile([P, 1], f32, tag=f"{nm}{eb}", name=f"{nm}{eb}")
                nc.gpsimd.dma_start(
                    out=t,
                    in_=bass.AP(
                        tensor=blob_d.tensor,
                        offset=spec[nm][0] + eb * P,
                        ap=[[1, P], [0, 1]],
                    ),
                )
                bias_t[(nm, eb)] = t
        # causal-mask machinery: iota(j - p) and per-(pos, s) thresholds
        iota_t = const.tile([P, QC], f32, tag="iota", name="iota")
        nc.gpsimd.iota(
            iota_t,
            pattern=[[1, QC]],
            base=0,
            channel_multiplier=-1,
            allow_small_or_imprecise_dtypes=True,
        )
        thr_all = const.tile([P, n_pos * 4], f32, tag="thr", name="thr")
        nc.gpsimd.dma_start(
            out=thr_all,
            in_=bass.AP(
                tensor=blob_d.tensor,
                offset=spec["cvec"][0],
                ap=[[0, P], [1, n_pos * 4]],
            ),
        )
        qoff_sb = const.tile([1, Lq // P], i32, tag="qoff", name="qoff")
        nc.gpsimd.dma_start(
            out=qoff_sb, in_=qoff_d.bitcast(i32).rearrange("(o n) -> o n", o=1)
        )

        # --- persistent big tensors ---
        KT = [big.tile([P, L], f32r, tag=f"KT{eb}", name=f"KT{eb}") for eb in range(EB)]
        QT = [
            big.tile([P, Lq], f32r, tag=f"QT{eb}", name=f"QT{eb}") for eb in range(EB)
        ]
        V = big.tile([P, NKB, E], f32r, tag="V", name="V")

        def transpose_128(src_sbuf, dst_sbuf, tpsum_pool):
            """dst = src.T for a [128,128] fp32 block, via PE + DVE copy."""
            pt = tpsum_pool.tile([P, P], f32, tag="tp", name="tp")
            nc.tensor.transpose(pt, src_sbuf, ident)
            nc.vector.tensor_copy(out=dst_sbuf, in_=pt)

        def wt_transpose(pool, tpsum, nm):
            wt = [
                pool.tile([P, E], f32r, tag=f"{nm}T{eb}", name=f"{nm}T{eb}")
                for eb in range(EB)
            ]
            for fb in range(EB):
                wn = pool.tile([P, E], f32, tag="wnat", name="wnat", bufs=2)
                nc.sync.dma_start(out=wn, in_=w_d[nm][fb * P : (fb + 1) * P, :])
                for eb in range(EB):
                    transpose_128(
                        wn[:, eb * P : (eb + 1) * P],
                        wt[eb][:, fb * P : (fb + 1) * P],
                        tpsum,
                    )
            return wt

        # --- phase 0+1a: W^T (k,v), then K^T and V over all key rows ---
        with ExitStack() as ph:
            wkv = ph.enter_context(tc.tile_pool(name="wkv", bufs=1))
            stage = ph.enter_context(tc.tile_pool(name="stage", bufs=3))
            xt = ph.enter_context(tc.tile_pool(name="xt", bufs=2))
            tpsum = ph.enter_context(tc.tile_pool(name="tpsum", bufs=2, space="PSUM"))
            ppsum = ph.enter_context(tc.tile_pool(name="ppsum", bufs=4, space="PSUM"))

            WT = {nm: wt_transpose(wkv, tpsum, nm) for nm in ("wk", "wv")}

            for ch in range(NCH):
                xtc = [
                    xt.tile([P, 512], f32r, tag=f"xt{eb}", name=f"xt{eb}")
                    for eb in range(EB)
                ]
                for lt in range(4):
                    xn = stage.tile([P, E], f32, tag="xnat", name="xnat")
                    r0 = ch * 512 + lt * P
                    nc.sync.dma_start(out=xn, in_=xk_d[r0 : r0 + P, :])
                    for eb in range(EB):
                        transpose_128(
                            xn[:, eb * P : (eb + 1) * P],
                            xtc[eb][:, lt * P : (lt + 1) * P],
                            tpsum,
                        )
                # K^T[:, this l-chunk]
                for eb in range(EB):
                    acc = ppsum.tile([P, 512], f32, tag="acc", name="acc")
                    for ein in range(EB):
                        nc.tensor.matmul(
                            acc,
                            WT["wk"][ein][:, eb * P : (eb + 1) * P],
                            xtc[ein],
                            start=(ein == 0),
                            stop=(ein == EB - 1),
                        )
                    nc.scalar.activation(
                        out=KT[eb][:, ch * 512 : (ch + 1) * 512],
                        in_=acc,
                        func=Ident,
                        bias=bias_t[("bk", eb)],
                        scale=1.0,
                    )
                # V rows of this l-chunk
                for lb in range(4):
                    acc = ppsum.tile([P, 512], f32, tag="acc", name="acc")
                    for ein in range(EB):
                        nc.tensor.matmul(
                            acc,
                            xtc[ein][:, lb * P : (lb + 1) * P],
                            WT["wv"][ein],
                            start=(ein == 0),
                            stop=(ein == EB - 1),
                        )
                    nc.vector.tensor_copy(out=V[:, ch * 4 + lb, :], in_=acc)

        # --- phase 1b: W^T (q), then Q^T over the gathered query rows ---
        with ExitStack() as ph:
            wqp = ph.enter_context(tc.tile_pool(name="wqp", bufs=1))
            stage = ph.enter_context(tc.tile_pool(name="stage2", bufs=2))
            xt = ph.enter_context(tc.tile_pool(name="xt2", bufs=1))
            tpsum = ph.enter_context(tc.tile_pool(name="tpsum2", bufs=2, space="PSUM"))
            ppsum = ph.enter_context(tc.tile_pool(name="ppsum2", bufs=4, space="PSUM"))

            WqT = wt_transpose(wqp, tpsum, "wq")

            for ch in range(Lq // 512):
                xtc = [
                    xt.tile([P, 512], f32r, tag=f"xq{eb}", name=f"xq{eb}")
                    for eb in range(EB)
                ]
                for lt in range(4):
                    xn = stage.tile([P, E], f32, tag="xqnat", name="xqnat")
                    ti = ch * 4 + lt
                    if DYNQ:
                        sv = nc.values_load(
                            qoff_sb[0:1, ti : ti + 1], min_val=0, max_val=L - P
                        )
                        nc.sync.dma_start(out=xn, in_=xk_d[ds(sv, P), :])
                    else:
                        xq_d = bpart("xq")
                        nc.sync.dma_start(out=xn, in_=xq_d[ti * P : (ti + 1) * P, :])
                    for eb in range(EB):
                        transpose_128(
                            xn[:, eb * P : (eb + 1) * P],
                            xtc[eb][:, lt * P : (lt + 1) * P],
                            tpsum,
                        )
                for eb in range(EB):
                    acc = ppsum.tile([P, 512], f32, tag="acc2", name="acc2")
                    for ein in range(EB):
                        nc.tensor.matmul(
                            acc,
                            WqT[ein][:, eb * P : (eb + 1) * P],
                            xtc[ein],
                            start=(ein == 0),
                            stop=(ein == EB - 1),
                        )
                    nc.scalar.activation(
                        out=QT[eb][:, ch * 512 : (ch + 1) * 512],
                        in_=acc,
                        func=Ident,
                        bias=bias_t[("bq", eb)],
                        scale=1.0,
                    )

        # --- phase 2: attention over chunk positions ---
        with ExitStack() as ph:
            spsum = ph.enter_context(tc.tile_pool(name="spsum", bufs=2, space="PSUM"))
            opsum = ph.enter_context(tc.tile_pool(name="opsum", bufs=1, space="PSUM"))
            dpsum = ph.enter_context(tc.tile_pool(name="dpsum", bufs=1, space="PSUM"))
            mpool = ph.enter_context(tc.tile_pool(name="mpool", bufs=2))
            ptp = ph.enter_context(tc.tile_pool(name="ptp", bufs=4))
            smp = ph.enter_context(tc.tile_pool(name="smp", bufs=2))
            opool = ph.enter_context(tc.tile_pool(name="opool", bufs=4))
            rpool = ph.enter_context(tc.tile_pool(name="rpool", bufs=4))

            for pos in range(n_pos):
                nb = KB[pos]
                q0 = pos * QC
                mct = mpool.tile([P, 4, QC], f32, tag="mct", name="mct")
                for s in range(4):
                    nc.vector.tensor_scalar(
                        out=mct[:, s, :],
                        in0=iota_t,
                        scalar1=thr_all[:, pos * 4 + s : pos * 4 + s + 1],
                        scalar2=NEG,
                        op0=mybir.AluOpType.is_lt,
                        op1=mybir.AluOpType.mult,
                    )
                o_ps = [
                    opsum.tile([P, E], f32, tag=f"o{qs}", name=f"o{qs}")
                    for qs in range(2)
                ]
                den = [
                    dpsum.tile([P, 2], f32, tag=f"den{qs}", name=f"den{qs}")
                    for qs in range(2)
                ]
                for kb in range(nb):
                    s_ps = spsum.tile([P, QC], f32, tag="s", name="s")
                    for ein in range(EB):
                        nc.tensor.matmul(
                            s_ps,
                            KT[ein][:, kb * P : (kb + 1) * P],
                            QT[ein][:, q0 : q0 + QC],
                            start=(ein == 0),
                            stop=(ein == EB - 1),
                        )
                    pt = ptp.tile([P, QC], f32r, tag="pt", name="pt")
                    if kb >= nb - 4:
                        sm = smp.tile([P, QC], f32, tag="sm", name="sm")
                        nc.vector.tensor_add(sm, s_ps, mct[:, kb - (nb - 4), :])
                        nc.scalar.activation(out=pt, in_=sm, func=Exp, scale=SCALE)
                    else:
                        nc.scalar.activation(out=pt, in_=s_ps, func=Exp, scale=SCALE)
                    for qs in range(2):
                        lhsT = pt[:, qs * P : (qs + 1) * P]
                        nc.tensor.matmul(
                            o_ps[qs],
                            lhsT,
                            V[:, kb, :],
                            start=(kb == 0),
                            stop=(kb == nb - 1),
                        )
                        nc.tensor.matmul(
                            den[qs],
                            lhsT,
                            ones,
                            start=(kb == 0),
                            stop=(kb == nb - 1),
                        )
                for qs in range(2):
                    rec = rpool.tile([P, 1], f32, tag="rec", name="rec")
                    nc.vector.reciprocal(rec, den[qs][:, 0:1])
                    osb = opool.tile([P, E], f32, tag="osb", name="osb")
                    nc.vector.tensor_scalar_mul(osb, o_ps[qs], rec)
                    nc.gpsimd.tensor_add(osb, osb, bv_bc)
                    r0 = q0 + qs * P
                    nc.sync.dma_start(out=yq_d[r0 : r0 + P, :], in_=osb)

    nc.compile()
    return nc


# ---------------------------------------------------------------------------
# cached-jit PJRT runner
# ---------------------------------------------------------------------------
class _Runner:
    def __init__(self, L):
        import jax
        from jax.experimental.shard_map import shard_map
        from jax.sharding import Mesh, NamedSharding, PartitionSpec

        import concourse.mybir as mybir
        from concourse import bass2jax

        self.jax = jax
        self.L = L
        nc = build_program(L)
        self.nc = nc
        bass2jax.install_neuronx_cc_hook()
        n_cores = 8
        partition_name = nc.partition_id_tensor.name if nc.partition_id_tensor else None
        in_names, out_names, out_avals, zero_outs = [], [], [], []
        for alloc in nc.m.functions[0].allocations:
            if not isinstance(alloc, mybir.MemoryLocationSet):
                continue
            name = alloc.memorylocations[0].name
            if alloc.kind == "ExternalInput":
                if name != partition_name:
                    in_names.append(name)
            elif alloc.kind == "ExternalOutput":
                out_names.append(name)
                shape = tuple(alloc.tensor_shape)
                dtype = mybir.dt.np(alloc.dtype)
                out_avals.append(jax.core.ShapedArray(shape, dtype))
                zero_outs.append(np.zeros(shape, dtype))
        self.in_names = in_names
        self.out_names = out_names
        all_in_names = list(in_names) + list(out_names)
        if partition_name is not None:
            all_in_names.append(partition_name)

        def _body(*args):
            operands = list(args)
            if partition_name is not None:
                operands.append(bass2jax.partition_id_tensor())
            outs = bass2jax._bass_exec_p.bind(
                *operands,
                out_avals=tuple(out_avals),
                in_names=tuple(all_in_names),
                out_names=tuple(out_names),
                lowering_input_output_aliases=(),
                sim_require_finite=True,
                sim_require_nnan=True,
                nc=nc,
            )
            return tuple(outs)

        devices = jax.devices()[:n_cores]
        mesh = Mesh(np.asarray(devices), ("core",))
        self.spec = NamedSharding(mesh, PartitionSpec("core"))
        n_params = len(in_names)
        donate = tuple(range(n_params, n_params + len(out_names)))
        self.fn = jax.jit(
            shard_map(
                _body,
                mesh=mesh,
                in_specs=(PartitionSpec("core"),) * (n_params + len(out_names)),
                out_specs=(PartitionSpec("core"),) * len(out_names),
                check_rep=False,
            ),
            donate_argnums=donate,
            keep_unused=True,
        )
        self._out_bufs = [
            np.zeros((n_cores * a.shape[0], *a.shape[1:]), a.dtype) for a in out_avals
        ]
        self._out_shapes = [a.shape for a in out_avals]
        self._build_redist(L, mesh, PartitionSpec, shard_map)

    def _build_redist(self, L, mesh, PartitionSpec, shard_map):
        """jit that assembles each core's input blob on device from a minimal
        upload: per-core batch halves (32MB total), 8-way-sharded weights, and
        tiny per-core index/threshold arrays."""
        import jax
        import jax.numpy as jnp

        spec_map, words = _blob_spec(L)
        ca, cb, KB = _chunk_layout(L)
        n_pos = len(KB)
        Lq = QC * n_pos
        perm = [(c, c ^ 1) for c in range(8)]
        w_words = 3 * E * E + 3 * E

        def cvec_for(chunks):
            c = np.zeros((n_pos, 4), np.float32)
            for i, t in enumerate(chunks):
                for s in range(4):
                    c[i, s] = float(128 * (KB[i] - 4 + s) - QC * t)
            return c.reshape(-1)

        def rows_for(chunks):
            return np.concatenate(
                [np.arange(QC * t, QC * t + QC) for t in chunks]
            ).astype(np.int32)

        def qoff_bits_for(chunks):
            q = np.zeros(len(chunks) * 2, np.int32)
            for i, t in enumerate(chunks):
                q[2 * i] = QC * t
                q[2 * i + 1] = QC * t + P
            return q

        self._cvec8 = np.stack([cvec_for(ca if c % 2 == 0 else cb) for c in range(8)])
        self._rows8 = np.stack([rows_for(ca if c % 2 == 0 else cb) for c in range(8)])
        self._qoff8 = np.stack(
            [qoff_bits_for(ca if c % 2 == 0 else cb) for c in range(8)]
        )
        self._wpad = ((w_words + 7) // 8) * 8

        def body(xs, ws, rws, cv, qo):
            xo = jax.lax.ppermute(xs, "core", perm=perm)
            half = jax.lax.axis_index("core") % 2
            a = jnp.concatenate([xs, xo], axis=0)
            b = jnp.concatenate([xo, xs], axis=0)
            xb = jnp.where(half == 0, a, b)
            wfull = jax.lax.all_gather(ws[0], "core", tiled=True)
            xq = jnp.take(xb, rws[0], axis=0)
            blob = jnp.concatenate(
                [
                    xb.reshape(-1),
                    wfull[:w_words],
                    cv[0],
                    qo[0].view(jnp.float32),
                    xq.reshape(-1),
                ]
            )
            assert blob.shape[0] == words, (blob.shape, words)
            return blob

        self.redist = jax.jit(
            shard_map(
                body,
                mesh=mesh,
                in_specs=(PartitionSpec("core"),) * 5,
                out_specs=PartitionSpec("core"),
                check_rep=False,
            )
        )

    def run_fast(self, x, Wq, bq, Wk, bk, Wv, bv):
        jax = self.jax
        L = self.L
        halves = np.concatenate(
            [x[c // 2, (c % 2) * (L // 2) : (c % 2 + 1) * (L // 2)] for c in range(8)]
        )
        wcat = np.concatenate(
            [
                np.asarray(Wq, np.float32).ravel(),
                np.asarray(Wk, np.float32).ravel(),
                np.asarray(Wv, np.float32).ravel(),
                np.asarray(bq, np.float32),
                np.asarray(bk, np.float32),
                np.asarray(bv, np.float32),
            ]
        )
        wcat = np.pad(wcat, (0, self._wpad - len(wcat))).reshape(8, -1)
        up = [
            jax.device_put(halves, self.spec),
            jax.device_put(wcat, self.spec),
            jax.device_put(self._rows8, self.spec),
            jax.device_put(self._cvec8, self.spec),
            jax.device_put(self._qoff8, self.spec),
        ]
        blob = self.redist(*up)
        outs = list(self.fn(blob, *self._out_bufs))
        host = [np.asarray(o) for o in outs]
        self._out_bufs = outs
        results = []
        for c in range(8):
            d = {}
            for i, nm in enumerate(self.out_names):
                sh = self._out_shapes[i]
                d[nm] = host[i].reshape(8, *sh)[c]
            results.append(d)
        return results

    def run(self, in_maps):
        jax = self.jax
        n_cores = len(in_maps)
        concat_in = [
            np.concatenate([np.asarray(in_maps[c][nm]) for c in range(n_cores)], axis=0)
            for nm in self.in_names
        ]
        dev_in = [jax.device_put(a, self.spec) for a in concat_in]
        outs = list(self.fn(*dev_in, *self._out_bufs))
        host = [np.asarray(o) for o in outs]
        self._out_bufs = outs  # donate previous outputs next call
        results = []
        for c in range(n_cores):
            d = {}
            for i, nm in enumerate(self.out_names):
                sh = self._out_shapes[i]
                d[nm] = host[i].reshape(n_cores, *sh)[c]
            results.append(d)
        return results


def kernel(x, Wq, bq, Wk, bk, Wv, bv):
    x = np.asarray(x, dtype=np.float32)
    B, L, _ = x.shape
    key = ("runner", L)
    if key not in _CACHE:
        _CACHE[key] = _Runner(L)
    runner = _CACHE[key]
    layout = _chunk_layout(L)[:2]
    if not _CACHE.get("no_fast"):
        try:
            results = runner.run_fast(x, Wq, bq, Wk, bk, Wv, bv)
            return _gather_outputs(results, layout, B, L)
        except Exception:
            _CACHE["no_fast"] = True
    in_maps, layout = _shard_inputs(x, Wq, bq, Wk, bk, Wv, bv, L)
    results = runner.run(in_maps)
    return _gather_outputs(results, layout, B, L)

